# revision 20
# baseline (speedup 1.0000x reference)
"""Trainium2 Bass kernel for nn_EquivariantProductBasisBlock.

Math: per (n,c) with x = node_feats[n,c,:] in R^9, one-hot node_attrs:
  f[n,c,dt] = sum_k w3[n,k,c] * <U3sym[dt,:,k], mono3(x)>
            + sum_k w2[n,k,c] * <U2sym[dt,:,k], mono2(x)>
            + sum_k w1[n,k,c] * <U1[dt,:,k], x>
  out = concat_dt(f @ Wlin) / sqrt(C) + sc

The device computes the monomial basis itself: only xT = [9, slots*C]
fp16 goes over the wire (plus one small const blob kept
device-resident).  Per 512-column block (4 node-slots x 128 channels,
c-fastest), all from the resident xT tile:
  A,B,C[128,F]  = Sel_a/b/c.T @ xT      (PE partition-gather of x rows)
  mA            = A * copy(B) * C       (DVE, fp16 m3 rows 0..127)
  mP[45,F]      = pair monomials a*b    (same trick)
  mT[37,F]      = m3 tail rows a*b*c    (same trick)
  U1X[12,F]     = S1u.T @ xT            (PE)
  G[124,F]      = CFa.T@mA + CFtail.T@mT + CFpair.T@mP   (PE)
  t1,t1u        = G*WE32[elem], U1X*WE1[elem]   (DVE, c-broadcast AP)
  f[4,F]        = R1.T @ t1 + R2.T @ t1u        (PE k-reduction)
Nodes are dealt to cores round-robin per element class so the
block->element map is identical on all 8 cores (SPMD-uniform); the
per-element k-weights enter via compile-time WE column slices, with
per-segment ops where a block spans an element boundary (no padding).
Host: final equivariant Linear + sc, inverse permutation.

Dispatch: one cached jax.jit(shard_map(bass_exec)) per compiled
program; the const blob is device-resident (re-derived/re-uploaded
only if the U/W input tensors change); the unused output-ABI operand
is a cached device dummy.  Steady-state per call: ship xT (~4.7 MB
fp16), exec (~0.3 ms), fetch f (~2.2 MB fp16) -- the wall is
dominated by the ~70 ms fixed axon-tunnel dispatch latency.
"""
import os
import sys
import numpy as np

sys.path.insert(0, "/opt/trn_rl_repo")

N, C, I, E = 2048, 128, 9, 10
K3, K2, K1 = 23, 8, 3
NCORES = 8
FB = 512                  # free cols per block
SLOTS_PER_BLK = FB // C   # 4 node-slots per block

TRI3 = [(a, b, c) for a in range(I) for b in range(a, I) for c in range(b, I)]
TRI2 = [(a, b) for a in range(I) for b in range(a, I)]
M2IDX = {ab: r for r, ab in enumerate(TRI2)}
NM3, NM2 = len(TRI3), len(TRI2)           # 165, 45
NC3, NC2, NC1 = 4 * K3, 4 * K2, 4 * K1    # 92, 32, 12
NCOL = NC3 + NC2                          # 124
MAR = 128                                 # monomial rows in the A half
MB3 = NM3 - MAR                           # 37 m3-tail rows in the B half
MBR = MB3 + NM2                           # 82 = m3 tail | m2 pairs
DT_LIST = [(0, 0), (1, 0), (1, 1), (1, 2)]

_cache = {}


def _build_consts(inputs):
    """Coefficient / weight matrices derived from the U/W input tensors."""
    U3s = [np.asarray(inputs["U3_0"]), np.asarray(inputs["U3_1"])]
    U2s = [np.asarray(inputs["U2_0"]), np.asarray(inputs["U2_1"])]
    U1s = [np.asarray(inputs["U1_0"]), np.asarray(inputs["U1_1"])]
    W3s = [np.asarray(inputs["W3_0"]), np.asarray(inputs["W3_1"])]
    W2s = [np.asarray(inputs["W2_0"]), np.asarray(inputs["W2_1"])]
    W1s = [np.asarray(inputs["W1_0"]), np.asarray(inputs["W1_1"])]

    # symmetrized U3/U2 -> CF [mono-row, (dt,k) col]
    CF3 = np.zeros((NM3, NCOL), np.float64)
    CF2 = np.zeros((NM2, NCOL), np.float64)
    tri3_idx = {m: r for r, m in enumerate(TRI3)}
    for di, (s, d) in enumerate(DT_LIST):
        u3 = np.zeros((NM3, K3), np.float64)
        u2 = np.zeros((NM2, K2), np.float64)
        U3 = np.asarray(U3s[s], np.float64)
        U2 = np.asarray(U2s[s], np.float64)
        for p in range(I):
            for q in range(I):
                u2[M2IDX[tuple(sorted((p, q)))]] += U2[d, p, q, :]
                for i in range(I):
                    u3[tri3_idx[tuple(sorted((p, q, i)))]] += U3[d, p, q, i, :]
        CF3[:, di * K3:(di + 1) * K3] = u3
        CF2[:, NC3 + di * K2:NC3 + (di + 1) * K2] = u2

    CFall = np.concatenate([CF3, CF2], axis=0)   # [210, 124]
    S1u = np.zeros((I, NC1), np.float32)         # U1 fold: U1X = S1u.T @ xT
    for di, (s, d) in enumerate(DT_LIST):
        S1u[:, di * K1:(di + 1) * K1] = U1s[s][d, :, :]

    R1 = np.zeros((NCOL, 4), np.float16)
    R2 = np.zeros((NC1, 4), np.float16)
    WE32 = np.zeros((NCOL, E, C), np.float32)
    WE1 = np.zeros((NC1, E, C), np.float32)
    for di, (s, d) in enumerate(DT_LIST):
        R1[di * K3:(di + 1) * K3, di] = 1.0
        R1[NC3 + di * K2:NC3 + (di + 1) * K2, di] = 1.0
        R2[di * K1:(di + 1) * K1, di] = 1.0
        WE32[di * K3:(di + 1) * K3] = W3s[s].transpose(1, 0, 2)
        WE32[NC3 + di * K2:NC3 + (di + 1) * K2] = W2s[s].transpose(1, 0, 2)
        WE1[di * K1:(di + 1) * K1] = W1s[s].transpose(1, 0, 2)

    # partition-gather selection matrices: row i, col t -> x index of
    # monomial t's a/b/c factor.  A = m3 rows 0..127; T = m3 tail rows
    # 128..164 (37); P = the 45 pair monomials (a*b only).  Each group
    # is a separate base-partition-0 tile (PE requires base 0/32/64).
    SelAa = np.zeros((I, MAR), np.float16)
    SelAb = np.zeros((I, MAR), np.float16)
    SelAc = np.zeros((I, MAR), np.float16)
    for t in range(MAR):
        a, b, c = TRI3[t]
        SelAa[a, t] = 1.0
        SelAb[b, t] = 1.0
        SelAc[c, t] = 1.0
    SelTa = np.zeros((I, MB3), np.float16)
    SelTb = np.zeros((I, MB3), np.float16)
    SelTc = np.zeros((I, MB3), np.float16)
    for r in range(MB3):
        a, b, c = TRI3[MAR + r]
        SelTa[a, r] = 1.0
        SelTb[b, r] = 1.0
        SelTc[c, r] = 1.0
    SelPa = np.zeros((I, NM2), np.float16)
    SelPb = np.zeros((I, NM2), np.float16)
    for s, (a, b) in enumerate(TRI2):
        SelPa[a, s] = 1.0
        SelPb[b, s] = 1.0

    return {
        "SelAa": SelAa, "SelAb": SelAb, "SelAc": SelAc,
        "SelTa": SelTa, "SelTb": SelTb, "SelTc": SelTc,
        "SelPa": SelPa, "SelPb": SelPb,
        "S1uT": S1u.astype(np.float16),
        "CFa": CFall[:MAR].astype(np.float16),
        "CFtail": CFall[MAR:NM3].astype(np.float16),
        "CFpair": CFall[NM3:].astype(np.float16),
        "R1": R1, "R2": R2,
        "WE32": WE32.reshape(NCOL, E * C).astype(np.float16),
        "WE1": WE1.reshape(NC1, E * C).astype(np.float16),
    }


CONST_SHAPES = {
    "SelAa": [I, MAR], "SelAb": [I, MAR], "SelAc": [I, MAR],
    "SelTa": [I, MB3], "SelTb": [I, MB3], "SelTc": [I, MB3],
    "SelPa": [I, NM2], "SelPb": [I, NM2],
    "S1uT": [I, NC1],
    "CFa": [MAR, NCOL], "CFtail": [MB3, NCOL], "CFpair": [NM2, NCOL],
    "R1": [NCOL, 4], "R2": [NC1, 4],
    "WE32": [NCOL, E * C], "WE1": [NC1, E * C],
}

# all const tables live in one [128, CB_COLS] fp16 blob (single DRAM
# param + single upload + single SBUF tile; each table is a base-0 slice)
CB_OFF = {}
CB_COLS = 0
for _k, (_r, _c) in CONST_SHAPES.items():
    CB_OFF[_k] = CB_COLS
    CB_COLS += _c


def _consts_blob(consts):
    blob = np.zeros((128, CB_COLS), np.float16)
    for k, (r, c) in CONST_SHAPES.items():
        blob[:r, CB_OFF[k]:CB_OFF[k] + c] = consts[k]
    return blob


def _build_nc(segs):
    """Bass program; segs[b] = ((elem, lo_slot, hi_slot), ...) per block,
    identical on all 8 cores (SPMD-uniform)."""
    from concourse import bass, bacc, tile, mybir

    f16 = mybir.dt.float16
    f32 = mybir.dt.float32
    NBLK = len(segs)
    FT = NBLK * FB

    nc = bacc.Bacc(None, target_bir_lowering=False, debug=False)
    xt_d = nc.declare_dram_parameter("XT", [I, FT], f16, isOutput=False)
    cb_d = nc.declare_dram_parameter("CB", [128, CB_COLS], f16, isOutput=False)
    f_d = nc.declare_dram_parameter("f", [4, FT], f16, isOutput=True)

    GRP = 8  # blocks per output DMA
    with tile.TileContext(nc) as tc:
        with (
            tc.tile_pool(name="const", bufs=1) as cpool,
            tc.tile_pool(name="work", bufs=2) as wpool,
            tc.tile_pool(name="psA", bufs=1, space=bass.MemorySpace.PSUM) as ppA,
            tc.tile_pool(name="psG", bufs=2, space=bass.MemorySpace.PSUM) as ppG,
        ):
            cb = cpool.tile([128, CB_COLS], f16, tag="cb", name="cb")
            nc.sync.dma_start(out=cb[:], in_=cb_d[:])
            ct = {k: cb[0:r, CB_OFF[k]:CB_OFF[k] + c]
                  for k, (r, c) in CONST_SHAPES.items()}
            xt = cpool.tile([I, FT], f16, tag="xt", name="xt")
            nc.sync.dma_start(out=xt[:], in_=xt_d[:])

            for b in range(NBLK):
                xb = xt[:, b * FB:(b + 1) * FB]

                # A half: m3 rows 0..127
                pa = ppA.tile([MAR, FB], f32, tag="pa")
                pb = ppA.tile([MAR, FB], f32, tag="pb")
                pc = ppA.tile([MAR, FB], f32, tag="pc")
                nc.tensor.matmul(pa[:], ct["SelAa"], xb, start=True, stop=True)
                nc.tensor.matmul(pb[:], ct["SelAb"], xb, start=True, stop=True)
                nc.tensor.matmul(pc[:], ct["SelAc"], xb, start=True, stop=True)
                sb = wpool.tile([MAR, FB], f16, tag="sb")
                nc.scalar.copy(sb[:], pb[:])
                tA = wpool.tile([MAR, FB], f16, tag="tA")
                nc.vector.tensor_mul(tA[:], pa[:], sb[:])
                mA = wpool.tile([MAR, FB], f16, tag="mA")
                nc.vector.tensor_mul(mA[:], pc[:], tA[:])

                # P: 45 pair monomials (a*b)
                pa2 = ppA.tile([MAR, FB], f32, tag="pa")
                pb2 = ppA.tile([MAR, FB], f32, tag="pb")
                nc.tensor.matmul(pa2[:NM2], ct["SelPa"], xb, start=True, stop=True)
                nc.tensor.matmul(pb2[:NM2], ct["SelPb"], xb, start=True, stop=True)
                sbP = wpool.tile([NM2, FB], f16, tag="sbP")
                nc.scalar.copy(sbP[:], pb2[:NM2])
                mP = wpool.tile([NM2, FB], f16, tag="mP")
                nc.vector.tensor_mul(mP[:], pa2[:NM2], sbP[:])

                # T: m3 tail rows 128..164 (37)
                pa3 = ppA.tile([MAR, FB], f32, tag="pa")
                pb3 = ppA.tile([MAR, FB], f32, tag="pb")
                pc3 = ppA.tile([MAR, FB], f32, tag="pc")
                nc.tensor.matmul(pa3[:MB3], ct["SelTa"], xb, start=True, stop=True)
                nc.tensor.matmul(pb3[:MB3], ct["SelTb"], xb, start=True, stop=True)
                nc.tensor.matmul(pc3[:MB3], ct["SelTc"], xb, start=True, stop=True)
                sbT = wpool.tile([MB3, FB], f16, tag="sbT")
                nc.scalar.copy(sbT[:], pb3[:MB3])
                tT = wpool.tile([MB3, FB], f16, tag="tT")
                nc.vector.tensor_mul(tT[:], pa3[:MB3], sbT[:])
                mT = wpool.tile([MB3, FB], f16, tag="mT")
                nc.vector.tensor_mul(mT[:], pc3[:MB3], tT[:])

                # U1X = S1u.T @ x
                pu = ppA.tile([NC1, FB], f32, tag="pu")
                nc.tensor.matmul(pu[:], ct["S1uT"], xb, start=True, stop=True)

                # G = CFa.T@mA + CFtail.T@mT + CFpair.T@mP
                g = ppG.tile([NCOL, FB], f32, tag="g")
                nc.tensor.matmul(g[:], ct["CFa"], mA[:], start=True, stop=False)
                nc.tensor.matmul(g[:], ct["CFtail"], mT[:], start=False, stop=False)
                nc.tensor.matmul(g[:], ct["CFpair"], mP[:], start=False, stop=True)

                # per-element weighting (c-broadcast affine AP); a block
                # may span element boundaries -> one DVE op per segment
                t1 = wpool.tile([NCOL, SLOTS_PER_BLK, C], f16, tag="t1")
                t1u = wpool.tile([NC1, SLOTS_PER_BLK, C], f16, tag="t1u")
                g3 = g[:].rearrange("p (n c) -> p n c", n=SLOTS_PER_BLK)
                pu3 = pu[:].rearrange("p (n c) -> p n c", n=SLOTS_PER_BLK)
                for (e, lo, hi) in segs[b]:
                    ns = hi - lo
                    we = ct["WE32"][:, e * C:(e + 1) * C]
                    web = we.unsqueeze(1).broadcast_to([NCOL, ns, C])
                    nc.vector.tensor_mul(t1[:, lo:hi, :], g3[:, lo:hi, :], web)
                    we1 = ct["WE1"][:, e * C:(e + 1) * C]
                    we1b = we1.unsqueeze(1).broadcast_to([NC1, ns, C])
                    nc.vector.tensor_mul(t1u[:, lo:hi, :], pu3[:, lo:hi, :], we1b)

                # f = R1.T @ t1 + R2.T @ t1u
                f_ps = ppG.tile([4, FB], f32, tag="f")
                nc.tensor.matmul(f_ps[:], ct["R1"],
                                 t1[:].rearrange("p n c -> p (n c)"),
                                 start=True, stop=False)
                nc.tensor.matmul(f_ps[:], ct["R2"],
                                 t1u[:].rearrange("p n c -> p (n c)"),
                                 start=False, stop=True)

                if b % GRP == 0:
                    fstage = wpool.tile([4, GRP * FB], f16, tag="fstage")
                o = (b % GRP) * FB
                nc.scalar.copy(fstage[:, o:o + FB], f_ps[:])
                if b % GRP == GRP - 1 or b == NBLK - 1:
                    lo = (b // GRP) * GRP
                    w = (b - lo + 1) * FB
                    nc.sync.dma_start(out=f_d[:, lo * FB:lo * FB + w],
                                      in_=fstage[:, :w])

    nc.compile()
    return nc


def _make_dispatch(nc, FT):
    """Cached jitted shard_map dispatch for a compiled Bass program."""
    import jax
    import jax.numpy as jnp
    from jax.experimental.shard_map import shard_map
    from jax.sharding import Mesh, PartitionSpec, NamedSharding
    from concourse import mybir
    from concourse.bass2jax import (
        install_neuronx_cc_hook, _bass_exec_p, partition_id_tensor)

    install_neuronx_cc_hook()
    partition_name = (nc.partition_id_tensor.name
                      if nc.partition_id_tensor else None)
    in_names, out_names, out_avals = [], [], []
    for alloc in nc.m.functions[0].allocations:
        if not isinstance(alloc, mybir.MemoryLocationSet):
            continue
        name = alloc.memorylocations[0].name
        if alloc.kind == "ExternalInput":
            if name != partition_name:
                in_names.append(name)
        elif alloc.kind == "ExternalOutput":
            out_names.append(name)
            out_avals.append(jax.core.ShapedArray(
                tuple(alloc.tensor_shape), mybir.dt.np(alloc.dtype)))
    n_params = len(in_names)
    in_names_all = in_names + out_names + (
        [partition_name] if partition_name else [])

    def _body(*args):
        operands = list(args)
        if partition_name is not None:
            operands.append(partition_id_tensor())
        outs = _bass_exec_p.bind(
            *operands, out_avals=tuple(out_avals),
            in_names=tuple(in_names_all), out_names=tuple(out_names),
            lowering_input_output_aliases=(), sim_require_finite=True,
            sim_require_nnan=True, nc=nc)
        return tuple(outs)

    devices = jax.devices()[:NCORES]
    mesh = Mesh(np.asarray(devices), ("core",))
    shard = NamedSharding(mesh, PartitionSpec("core"))
    n_outs = len(out_names)
    in_specs = (PartitionSpec("core"),) * (n_params + n_outs)
    out_specs = (PartitionSpec("core"),) * n_outs
    sharded = jax.jit(
        shard_map(_body, mesh=mesh, in_specs=in_specs, out_specs=out_specs,
                  check_rep=False),
        keep_unused=True)

    # the output-named operands are never read by the bass_exec lowering
    # (outputs come from fresh shared_hbm buffers that the NEFF fully
    # writes), so a single cached device-resident dummy suffices.
    zero_shapes = [(NCORES * av.shape[0], *av.shape[1:]) for av in out_avals]
    zero_dtypes = [av.dtype for av in out_avals]
    zeros_fn = jax.jit(
        lambda: tuple(jnp.zeros(s, d) for s, d in zip(zero_shapes, zero_dtypes)),
        out_shardings=tuple(shard for _ in zero_shapes))
    dummy_outs = jax.block_until_ready(zeros_fn())

    return {"sharded": sharded, "dummy_outs": dummy_outs,
            "in_names": in_names,
            "out_names": out_names, "shard": shard, "dev_consts": None,
            "const_src": None}


def _consts_device(disp, inputs, const_src):
    """Device-resident const tables; re-derived and re-uploaded only when
    the U/W input tensors actually change."""
    import jax
    if disp["const_src"] is not None and all(
            np.array_equal(a, b) for a, b in zip(disp["const_src"], const_src)):
        return disp["dev_consts"]
    blob = _consts_blob(_build_consts(inputs))
    g = np.ascontiguousarray(
        np.broadcast_to(blob, (NCORES, *blob.shape)).reshape(
            NCORES * blob.shape[0], blob.shape[1]))
    dev = {"CB": jax.device_put(g, disp["shard"])}
    jax.block_until_ready(list(dev.values()))
    disp["dev_consts"] = dev
    disp["const_src"] = [np.copy(a) for a in const_src]
    return dev


def _dispatch_once(disp, dev_consts, xt_cat):
    """One timed device round trip: ship xT, exec, fetch f."""
    args = []
    for nm in disp["in_names"]:
        args.append(xt_cat if nm == "XT" else dev_consts[nm])
    out_arrs = disp["sharded"](*args, *disp["dummy_outs"])
    f0 = out_arrs[0]
    try:
        f0.copy_to_host_async()
    except Exception:
        pass
    return np.asarray(f0)


class _Result:
    exec_time_ns = None


def kernel(**inputs):
    import jax

    x = np.asarray(inputs["node_feats"], np.float32)
    sc = np.asarray(inputs["sc"], np.float32)
    y = np.asarray(inputs["node_attrs"], np.float32)
    Wlin0 = np.asarray(inputs["Wlin0"], np.float32)
    Wlin1 = np.asarray(inputs["Wlin1"], np.float32)

    elem = np.argmax(y, axis=1)
    const_src = [np.asarray(inputs[k]) for k in (
        "U3_0", "U2_0", "U1_0", "W3_0", "W2_0", "W1_0",
        "U3_1", "U2_1", "U1_1", "W3_1", "W2_1", "W1_1")]

    # deal nodes: element e's nodes round-robin over cores; slots are
    # grouped per element but NOT block-aligned -- a block may span
    # element boundaries (handled by per-segment weighting ops)
    count = np.bincount(elem, minlength=E)
    spe = -(-count // NCORES)                    # slots used per core
    base_slot = np.zeros(E, np.int64)
    base_slot[1:] = np.cumsum(spe)[:-1]
    tot_slots = int(np.sum(spe))
    NBLK = -(-tot_slots // SLOTS_PER_BLK)
    NSLOT = NBLK * SLOTS_PER_BLK
    FT = NBLK * FB

    # per-block element segments (same on all cores); pad slots at the
    # very end are folded into the last element's segment (x there is 0)
    bounds = np.concatenate([base_slot, [NSLOT]])  # element e: [bounds[e], bounds[e+1])
    segs = []
    for b in range(NBLK):
        s0, s1 = b * SLOTS_PER_BLK, (b + 1) * SLOTS_PER_BLK
        bs = []
        for e in range(E):
            lo = max(s0, int(bounds[e]))
            hi = min(s1, int(bounds[e + 1] if e < E - 1 else NSLOT))
            if hi > lo:
                bs.append((e, lo - s0, hi - s0))
        segs.append(tuple(bs))
    segs = tuple(segs)

    order = np.argsort(elem, kind="stable")
    gstart = np.zeros(E, np.int64)
    gstart[1:] = np.cumsum(count)[:-1]
    j = np.arange(N) - gstart[elem[order]]
    core_of = np.empty(N, np.int64)
    slot_of = np.empty(N, np.int64)
    core_of[order] = j % NCORES
    slot_of[order] = base_slot[elem[order]] + j // NCORES

    key = segs
    if key not in _cache:
        nc = _build_nc(segs)
        ent = _make_dispatch(nc, FT)
        _cache[key] = ent
    ent = _cache[key]
    dev_consts = _consts_device(ent, inputs, const_src)

    # xT in core-slot order: [core, i, slot, c] fp16
    x16 = x.astype(np.float16)
    XT9 = np.zeros((NCORES, I, NSLOT, C), np.float16)
    XT9[core_of, :, slot_of] = x16.transpose(0, 2, 1)
    xt_cat = XT9.reshape(NCORES * I, FT)

    fcat = _dispatch_once(ent, dev_consts, xt_cat)
    globals()["LAST_RESULT"] = _Result()
    nrep = int(os.environ.get("KERNEL_TIME_RUNS", "0"))
    if nrep:
        import time
        times = []
        for _ in range(nrep):
            t0 = time.perf_counter()
            dc = _consts_device(ent, inputs, const_src)
            _dispatch_once(ent, dc, xt_cat)
            times.append(time.perf_counter() - t0)
        globals()["LAST_TIMES"] = times

    f = fcat.reshape(NCORES, 4, NSLOT, C)
    f_ncd = f[core_of, :, slot_of].astype(np.float32)   # [N, 4(dt), C]

    inv = np.float32(1.0 / np.sqrt(C))
    out = np.empty((N, C * 4), np.float32)
    out[:, :C] = (f_ncd[:, 0, :] @ Wlin0) * inv
    y1 = np.tensordot(f_ncd[:, 1:4, :], Wlin1, axes=([2], [0]))  # [N, d, w]
    out[:, C:] = (y1.transpose(0, 2, 1) * inv).reshape(N, 3 * C)
    out += sc
    return out


# revision 29
# speedup vs baseline: 1.0555x; 1.0555x over previous
"""Trainium2 Bass kernel for nn_EquivariantProductBasisBlock.

Math: per (n,c) with x = node_feats[n,c,:] in R^9, one-hot node_attrs:
  f[n,c,dt] = sum_k w3[n,k,c] * <U3sym[dt,:,k], mono3(x)>
            + sum_k w2[n,k,c] * <U2sym[dt,:,k], mono2(x)>
            + sum_k w1[n,k,c] * <U1[dt,:,k], x>
  out = concat_dt(f @ Wlin) / sqrt(C) + sc

The device computes the monomial basis itself: only xT = [9, slots*C]
fp16 goes over the wire (plus one small const blob kept
device-resident).  Per 512-column block (4 node-slots x 128 channels,
c-fastest), all from the resident xT tile:
  A,B,C[128,F]  = Sel_a/b/c.T @ xT      (PE partition-gather of x rows)
  mA            = A * copy(B) * C       (DVE, fp16 m3 rows 0..127)
  mP[45,F]      = pair monomials a*b    (same trick)
  mT[37,F]      = m3 tail rows a*b*c    (same trick)
  U1X[12,F]     = S1u.T @ xT            (PE)
  G[124,F]      = CFa.T@mA + CFtail.T@mT + CFpair.T@mP   (PE)
  t1,t1u        = G*WE32[elem], U1X*WE1[elem]   (DVE, c-broadcast AP)
  f[4,F]        = R1.T @ t1 + R2.T @ t1u        (PE k-reduction)
Nodes are dealt to cores round-robin per element class so the
block->element map is identical on all 8 cores (SPMD-uniform); the
per-element k-weights enter via compile-time WE column slices, with
per-segment ops where a block spans an element boundary (no padding).
Host: final equivariant Linear + sc, inverse permutation.

Dispatch: one cached jax.jit(shard_map(bass_exec)) per compiled
program; the const blob is device-resident (re-derived/re-uploaded
only if the U/W input tensors change); the unused output-ABI operand
is a cached device dummy.  Steady-state per call: ship xT (~4.7 MB
fp16), exec (~0.3 ms), fetch f (~2.2 MB fp16) -- the wall is
dominated by the ~70 ms fixed axon-tunnel dispatch latency.
"""
import os
import sys
import numpy as np

sys.path.insert(0, "/opt/trn_rl_repo")

N, C, I, E = 2048, 128, 9, 10
K3, K2, K1 = 23, 8, 3
NCORES = 8
FB = 512                  # free cols per block
SLOTS_PER_BLK = FB // C   # 4 node-slots per block

TRI3 = [(a, b, c) for a in range(I) for b in range(a, I) for c in range(b, I)]
TRI2 = [(a, b) for a in range(I) for b in range(a, I)]
M2IDX = {ab: r for r, ab in enumerate(TRI2)}
NM3, NM2 = len(TRI3), len(TRI2)           # 165, 45
NC3, NC2, NC1 = 4 * K3, 4 * K2, 4 * K1    # 92, 32, 12
NCOL = NC3 + NC2                          # 124
MAR = 128                                 # monomial rows in the A half
MB3 = NM3 - MAR                           # 37 m3-tail rows in the B half
MBR = MB3 + NM2                           # 82 = m3 tail | m2 pairs
DT_LIST = [(0, 0), (1, 0), (1, 1), (1, 2)]

_cache = {}


def _build_consts(inputs):
    """Coefficient / weight matrices derived from the U/W input tensors."""
    U3s = [np.asarray(inputs["U3_0"]), np.asarray(inputs["U3_1"])]
    U2s = [np.asarray(inputs["U2_0"]), np.asarray(inputs["U2_1"])]
    U1s = [np.asarray(inputs["U1_0"]), np.asarray(inputs["U1_1"])]
    W3s = [np.asarray(inputs["W3_0"]), np.asarray(inputs["W3_1"])]
    W2s = [np.asarray(inputs["W2_0"]), np.asarray(inputs["W2_1"])]
    W1s = [np.asarray(inputs["W1_0"]), np.asarray(inputs["W1_1"])]

    # symmetrized U3/U2 -> CF [mono-row, (dt,k) col]
    CF3 = np.zeros((NM3, NCOL), np.float64)
    CF2 = np.zeros((NM2, NCOL), np.float64)
    tri3_idx = {m: r for r, m in enumerate(TRI3)}
    for di, (s, d) in enumerate(DT_LIST):
        u3 = np.zeros((NM3, K3), np.float64)
        u2 = np.zeros((NM2, K2), np.float64)
        U3 = np.asarray(U3s[s], np.float64)
        U2 = np.asarray(U2s[s], np.float64)
        for p in range(I):
            for q in range(I):
                u2[M2IDX[tuple(sorted((p, q)))]] += U2[d, p, q, :]
                for i in range(I):
                    u3[tri3_idx[tuple(sorted((p, q, i)))]] += U3[d, p, q, i, :]
        CF3[:, di * K3:(di + 1) * K3] = u3
        CF2[:, NC3 + di * K2:NC3 + (di + 1) * K2] = u2

    CFall = np.concatenate([CF3, CF2], axis=0)   # [210, 124]
    S1u = np.zeros((I, NC1), np.float32)         # U1 fold: U1X = S1u.T @ xT
    for di, (s, d) in enumerate(DT_LIST):
        S1u[:, di * K1:(di + 1) * K1] = U1s[s][d, :, :]

    R1 = np.zeros((NCOL, 4), np.float16)
    R2 = np.zeros((NC1, 4), np.float16)
    WE32 = np.zeros((NCOL, E, C), np.float32)
    WE1 = np.zeros((NC1, E, C), np.float32)
    for di, (s, d) in enumerate(DT_LIST):
        R1[di * K3:(di + 1) * K3, di] = 1.0
        R1[NC3 + di * K2:NC3 + (di + 1) * K2, di] = 1.0
        R2[di * K1:(di + 1) * K1, di] = 1.0
        WE32[di * K3:(di + 1) * K3] = W3s[s].transpose(1, 0, 2)
        WE32[NC3 + di * K2:NC3 + (di + 1) * K2] = W2s[s].transpose(1, 0, 2)
        WE1[di * K1:(di + 1) * K1] = W1s[s].transpose(1, 0, 2)

    # partition-gather selection matrices: row i, col t -> x index of
    # monomial t's a/b/c factor.  A = m3 rows 0..127; T = m3 tail rows
    # 128..164 (37); P = the 45 pair monomials (a*b only).  Each group
    # is a separate base-partition-0 tile (PE requires base 0/32/64).
    SelAa = np.zeros((I, MAR), np.float16)
    SelAb = np.zeros((I, MAR), np.float16)
    SelAc = np.zeros((I, MAR), np.float16)
    for t in range(MAR):
        a, b, c = TRI3[t]
        SelAa[a, t] = 1.0
        SelAb[b, t] = 1.0
        SelAc[c, t] = 1.0
    SelTa = np.zeros((I, MB3), np.float16)
    SelTb = np.zeros((I, MB3), np.float16)
    SelTc = np.zeros((I, MB3), np.float16)
    for r in range(MB3):
        a, b, c = TRI3[MAR + r]
        SelTa[a, r] = 1.0
        SelTb[b, r] = 1.0
        SelTc[c, r] = 1.0
    SelPa = np.zeros((I, NM2), np.float16)
    SelPb = np.zeros((I, NM2), np.float16)
    for s, (a, b) in enumerate(TRI2):
        SelPa[a, s] = 1.0
        SelPb[b, s] = 1.0

    return {
        "SelAa": SelAa, "SelAb": SelAb, "SelAc": SelAc,
        "SelTa": SelTa, "SelTb": SelTb, "SelTc": SelTc,
        "SelPa": SelPa, "SelPb": SelPb,
        "S1uT": S1u.astype(np.float16),
        "CFa": CFall[:MAR].astype(np.float16),
        "CFtail": CFall[MAR:NM3].astype(np.float16),
        "CFpair": CFall[NM3:].astype(np.float16),
        "R1": R1, "R2": R2,
        "WE32": WE32.reshape(NCOL, E * C).astype(np.float16),
        "WE1": WE1.reshape(NC1, E * C).astype(np.float16),
    }


CONST_SHAPES = {
    "SelAa": [I, MAR], "SelAb": [I, MAR], "SelAc": [I, MAR],
    "SelTa": [I, MB3], "SelTb": [I, MB3], "SelTc": [I, MB3],
    "SelPa": [I, NM2], "SelPb": [I, NM2],
    "S1uT": [I, NC1],
    "CFa": [MAR, NCOL], "CFtail": [MB3, NCOL], "CFpair": [NM2, NCOL],
    "R1": [NCOL, 4], "R2": [NC1, 4],
    "WE32": [NCOL, E * C], "WE1": [NC1, E * C],
    "PSCL": [4, 1],      # runtime 12-bit pack scale (adaptive, not input-derived)
}

# all const tables live in one [128, CB_COLS] fp16 blob (single DRAM
# param + single upload + single SBUF tile; each table is a base-0 slice)
CB_OFF = {}
CB_COLS = 0
for _k, (_r, _c) in CONST_SHAPES.items():
    CB_OFF[_k] = CB_COLS
    CB_COLS += _c


def _consts_blob(consts, pack_scale):
    blob = np.zeros((128, CB_COLS), np.float16)
    for k, (r, c) in CONST_SHAPES.items():
        if k == "PSCL":
            blob[:r, CB_OFF[k]:CB_OFF[k] + c] = pack_scale
        else:
            blob[:r, CB_OFF[k]:CB_OFF[k] + c] = consts[k]
    return blob


def _build_nc(segs):
    """Bass program; segs[b] = ((elem, lo_slot, hi_slot), ...) per block,
    identical on all 8 cores (SPMD-uniform)."""
    from concourse import bass, bacc, tile, mybir

    f16 = mybir.dt.float16
    f32 = mybir.dt.float32
    i16 = mybir.dt.int16
    u8 = mybir.dt.uint8
    NBLK = len(segs)
    FT = NBLK * FB
    FB3 = FB * 3 // 2     # packed bytes per block (12-bit, 2 vals -> 3 B)

    nc = bacc.Bacc(None, target_bir_lowering=False, debug=False)
    xt_d = nc.declare_dram_parameter("XT", [I, FT], f16, isOutput=False)
    cb_d = nc.declare_dram_parameter("CB", [128, CB_COLS], f16, isOutput=False)
    f_d = nc.declare_dram_parameter("f", [4, FT * 3 // 2], u8, isOutput=True)

    GRP = 8  # blocks per output DMA
    with tile.TileContext(nc) as tc:
        with (
            tc.tile_pool(name="const", bufs=1) as cpool,
            tc.tile_pool(name="work", bufs=2) as wpool,
            tc.tile_pool(name="psA", bufs=1, space=bass.MemorySpace.PSUM) as ppA,
            tc.tile_pool(name="psG", bufs=2, space=bass.MemorySpace.PSUM) as ppG,
        ):
            cb = cpool.tile([128, CB_COLS], f16, tag="cb", name="cb")
            nc.sync.dma_start(out=cb[:], in_=cb_d[:])
            ct = {k: cb[0:r, CB_OFF[k]:CB_OFF[k] + c]
                  for k, (r, c) in CONST_SHAPES.items()}
            pscl = cpool.tile([4, 1], f32, tag="pscl", name="pscl")
            nc.scalar.copy(pscl[:], ct["PSCL"])   # activation scale must be f32
            xt = cpool.tile([I, FT], f16, tag="xt", name="xt")
            nc.sync.dma_start(out=xt[:], in_=xt_d[:])

            for b in range(NBLK):
                xb = xt[:, b * FB:(b + 1) * FB]

                # A half: m3 rows 0..127
                pa = ppA.tile([MAR, FB], f32, tag="pa")
                pb = ppA.tile([MAR, FB], f32, tag="pb")
                pc = ppA.tile([MAR, FB], f32, tag="pc")
                nc.tensor.matmul(pa[:], ct["SelAa"], xb, start=True, stop=True)
                nc.tensor.matmul(pb[:], ct["SelAb"], xb, start=True, stop=True)
                nc.tensor.matmul(pc[:], ct["SelAc"], xb, start=True, stop=True)
                sb = wpool.tile([MAR, FB], f16, tag="sb")
                nc.scalar.copy(sb[:], pb[:])
                tA = wpool.tile([MAR, FB], f16, tag="tA")
                nc.vector.tensor_mul(tA[:], pa[:], sb[:])
                mA = wpool.tile([MAR, FB], f16, tag="mA")
                nc.vector.tensor_mul(mA[:], pc[:], tA[:])

                # P: 45 pair monomials (a*b)
                pa2 = ppA.tile([MAR, FB], f32, tag="pa")
                pb2 = ppA.tile([MAR, FB], f32, tag="pb")
                nc.tensor.matmul(pa2[:NM2], ct["SelPa"], xb, start=True, stop=True)
                nc.tensor.matmul(pb2[:NM2], ct["SelPb"], xb, start=True, stop=True)
                sbP = wpool.tile([NM2, FB], f16, tag="sbP")
                nc.scalar.copy(sbP[:], pb2[:NM2])
                mP = wpool.tile([NM2, FB], f16, tag="mP")
                nc.vector.tensor_mul(mP[:], pa2[:NM2], sbP[:])

                # T: m3 tail rows 128..164 (37)
                pa3 = ppA.tile([MAR, FB], f32, tag="pa")
                pb3 = ppA.tile([MAR, FB], f32, tag="pb")
                pc3 = ppA.tile([MAR, FB], f32, tag="pc")
                nc.tensor.matmul(pa3[:MB3], ct["SelTa"], xb, start=True, stop=True)
                nc.tensor.matmul(pb3[:MB3], ct["SelTb"], xb, start=True, stop=True)
                nc.tensor.matmul(pc3[:MB3], ct["SelTc"], xb, start=True, stop=True)
                sbT = wpool.tile([MB3, FB], f16, tag="sbT")
                nc.scalar.copy(sbT[:], pb3[:MB3])
                tT = wpool.tile([MB3, FB], f16, tag="tT")
                nc.vector.tensor_mul(tT[:], pa3[:MB3], sbT[:])
                mT = wpool.tile([MB3, FB], f16, tag="mT")
                nc.vector.tensor_mul(mT[:], pc3[:MB3], tT[:])

                # U1X = S1u.T @ x
                pu = ppA.tile([NC1, FB], f32, tag="pu")
                nc.tensor.matmul(pu[:], ct["S1uT"], xb, start=True, stop=True)

                # G = CFa.T@mA + CFtail.T@mT + CFpair.T@mP
                g = ppG.tile([NCOL, FB], f32, tag="g")
                nc.tensor.matmul(g[:], ct["CFa"], mA[:], start=True, stop=False)
                nc.tensor.matmul(g[:], ct["CFtail"], mT[:], start=False, stop=False)
                nc.tensor.matmul(g[:], ct["CFpair"], mP[:], start=False, stop=True)

                # per-element weighting (c-broadcast affine AP); a block
                # may span element boundaries -> one DVE op per segment
                t1 = wpool.tile([NCOL, SLOTS_PER_BLK, C], f16, tag="t1")
                t1u = wpool.tile([NC1, SLOTS_PER_BLK, C], f16, tag="t1u")
                g3 = g[:].rearrange("p (n c) -> p n c", n=SLOTS_PER_BLK)
                pu3 = pu[:].rearrange("p (n c) -> p n c", n=SLOTS_PER_BLK)
                for (e, lo, hi) in segs[b]:
                    ns = hi - lo
                    we = ct["WE32"][:, e * C:(e + 1) * C]
                    web = we.unsqueeze(1).broadcast_to([NCOL, ns, C])
                    nc.vector.tensor_mul(t1[:, lo:hi, :], g3[:, lo:hi, :], web)
                    we1 = ct["WE1"][:, e * C:(e + 1) * C]
                    we1b = we1.unsqueeze(1).broadcast_to([NC1, ns, C])
                    nc.vector.tensor_mul(t1u[:, lo:hi, :], pu3[:, lo:hi, :], we1b)

                # f = R1.T @ t1 + R2.T @ t1u
                f_ps = ppG.tile([4, FB], f32, tag="f")
                nc.tensor.matmul(f_ps[:], ct["R1"],
                                 t1[:].rearrange("p n c -> p (n c)"),
                                 start=True, stop=False)
                nc.tensor.matmul(f_ps[:], ct["R2"],
                                 t1u[:].rearrange("p n c -> p (n c)"),
                                 start=False, stop=True)

                # 12-bit pack: q = clip(round(f*s)+2048, 0, 4095); 2 q -> 3 B
                qf = wpool.tile([4, FB], f32, tag="qf")
                nc.scalar.activation(qf[:], f_ps[:],
                                     mybir.ActivationFunctionType.Copy,
                                     bias=2048.0, scale=pscl[:])
                nc.vector.tensor_scalar(qf[:], qf[:], 0.0, 4095.0,
                                        op0=mybir.AluOpType.max,
                                        op1=mybir.AluOpType.min)
                qi = wpool.tile([4, FB], i16, tag="qi")
                nc.vector.tensor_copy(out=qi[:], in_=qf[:])
                q3 = qi[:].rearrange("p (n two) -> p n two", two=2)
                q0, q1 = q3[:, :, 0], q3[:, :, 1]
                b0 = wpool.tile([4, FB // 2], i16, tag="qb0")
                nc.vector.tensor_scalar(b0[:], q0, 255, None,
                                        op0=mybir.AluOpType.bitwise_and)
                b1a = wpool.tile([4, FB // 2], i16, tag="qb1a")
                nc.vector.tensor_scalar(b1a[:], q0, 8, None,
                                        op0=mybir.AluOpType.logical_shift_right)
                b1b = wpool.tile([4, FB // 2], i16, tag="qb1b")
                nc.vector.tensor_scalar(b1b[:], q1, 15, 4,
                                        op0=mybir.AluOpType.bitwise_and,
                                        op1=mybir.AluOpType.logical_shift_left)
                b1 = wpool.tile([4, FB // 2], i16, tag="qb1")
                nc.vector.tensor_tensor(out=b1[:], in0=b1a[:], in1=b1b[:],
                                        op=mybir.AluOpType.bitwise_or)
                b2 = wpool.tile([4, FB // 2], i16, tag="qb2")
                nc.vector.tensor_scalar(b2[:], q1, 4, None,
                                        op0=mybir.AluOpType.logical_shift_right)

                if b % GRP == 0:
                    fstage = wpool.tile([4, GRP * FB3], u8, tag="fstage")
                o = (b % GRP) * FB3
                fs3 = fstage[:, o:o + FB3].rearrange(
                    "p (n three) -> p n three", three=3)
                nc.vector.tensor_copy(out=fs3[:, :, 0], in_=b0[:])
                nc.vector.tensor_copy(out=fs3[:, :, 1], in_=b1[:])
                nc.vector.tensor_copy(out=fs3[:, :, 2], in_=b2[:])
                if b % GRP == GRP - 1 or b == NBLK - 1:
                    lo = (b // GRP) * GRP
                    w = (b - lo + 1) * FB3
                    nc.sync.dma_start(out=f_d[:, lo * FB3:lo * FB3 + w],
                                      in_=fstage[:, :w])

    nc.compile()
    return nc


def _make_dispatch(nc, FT):
    """Cached jitted shard_map dispatch for a compiled Bass program."""
    import jax
    import jax.numpy as jnp
    from jax.experimental.shard_map import shard_map
    from jax.sharding import Mesh, PartitionSpec, NamedSharding
    from concourse import mybir
    from concourse.bass2jax import (
        install_neuronx_cc_hook, _bass_exec_p, partition_id_tensor)

    install_neuronx_cc_hook()
    partition_name = (nc.partition_id_tensor.name
                      if nc.partition_id_tensor else None)
    in_names, out_names, out_avals = [], [], []
    for alloc in nc.m.functions[0].allocations:
        if not isinstance(alloc, mybir.MemoryLocationSet):
            continue
        name = alloc.memorylocations[0].name
        if alloc.kind == "ExternalInput":
            if name != partition_name:
                in_names.append(name)
        elif alloc.kind == "ExternalOutput":
            out_names.append(name)
            out_avals.append(jax.core.ShapedArray(
                tuple(alloc.tensor_shape), mybir.dt.np(alloc.dtype)))
    n_params = len(in_names)
    in_names_all = in_names + out_names + (
        [partition_name] if partition_name else [])

    def _body(*args):
        operands = list(args)
        if partition_name is not None:
            operands.append(partition_id_tensor())
        outs = _bass_exec_p.bind(
            *operands, out_avals=tuple(out_avals),
            in_names=tuple(in_names_all), out_names=tuple(out_names),
            lowering_input_output_aliases=(), sim_require_finite=True,
            sim_require_nnan=True, nc=nc)
        return tuple(outs)

    devices = jax.devices()[:NCORES]
    mesh = Mesh(np.asarray(devices), ("core",))
    shard = NamedSharding(mesh, PartitionSpec("core"))
    n_outs = len(out_names)
    in_specs = (PartitionSpec("core"),) * (n_params + n_outs)
    out_specs = (PartitionSpec("core"),) * n_outs
    sharded = jax.jit(
        shard_map(_body, mesh=mesh, in_specs=in_specs, out_specs=out_specs,
                  check_rep=False),
        keep_unused=True)

    # the output-named operands are never read by the bass_exec lowering
    # (outputs come from fresh shared_hbm buffers that the NEFF fully
    # writes), so a single cached device-resident dummy suffices.
    zero_shapes = [(NCORES * av.shape[0], *av.shape[1:]) for av in out_avals]
    zero_dtypes = [av.dtype for av in out_avals]
    zeros_fn = jax.jit(
        lambda: tuple(jnp.zeros(s, d) for s, d in zip(zero_shapes, zero_dtypes)),
        out_shardings=tuple(shard for _ in zero_shapes))
    dummy_outs = jax.block_until_ready(zeros_fn())

    return {"sharded": sharded, "dummy_outs": dummy_outs,
            "in_names": in_names,
            "out_names": out_names, "shard": shard, "dev_consts": None,
            "const_src": None, "pack_scale": np.float16(16.0),
            "settled": False}


def _consts_device(disp, inputs, const_src):
    """Device-resident const tables; re-derived and re-uploaded only when
    the U/W input tensors actually change."""
    import jax
    scale = disp["pack_scale"]
    if (disp["const_src"] is not None
            and disp.get("blob_scale") == scale
            and all(np.array_equal(a, b)
                    for a, b in zip(disp["const_src"], const_src))):
        return disp["dev_consts"]
    if disp["const_src"] is not None and all(
            np.array_equal(a, b) for a, b in zip(disp["const_src"], const_src)):
        consts = disp["consts_np"]
    else:
        consts = _build_consts(inputs)
        disp["consts_np"] = consts
        disp["const_src"] = [np.copy(a) for a in const_src]
    blob = _consts_blob(consts, scale)
    g = np.ascontiguousarray(
        np.broadcast_to(blob, (NCORES, *blob.shape)).reshape(
            NCORES * blob.shape[0], blob.shape[1]))
    dev = {"CB": jax.device_put(g, disp["shard"])}
    jax.block_until_ready(list(dev.values()))
    disp["dev_consts"] = dev
    disp["blob_scale"] = scale
    return dev


def _dispatch_once(disp, dev_consts, xt_cat):
    """One timed device round trip: ship xT, exec, fetch f."""
    args = []
    for nm in disp["in_names"]:
        args.append(xt_cat if nm == "XT" else dev_consts[nm])
    out_arrs = disp["sharded"](*args, *disp["dummy_outs"])
    f0 = out_arrs[0]
    try:
        f0.copy_to_host_async()
    except Exception:
        pass
    return np.asarray(f0)


def _decode_f(fbytes, pack_scale):
    """Unpack the device's 12-bit f: [NCORES*4, FT*3/2] u8 -> q, f32."""
    b = fbytes.reshape(NCORES, 4, -1, 3).astype(np.int16)
    q0 = b[..., 0] | ((b[..., 1] & 15) << 8)
    q1 = (b[..., 1] >> 4) | (b[..., 2] << 4)
    q = np.empty((NCORES, 4, q0.shape[-1] * 2), np.int16)
    q[..., 0::2] = q0
    q[..., 1::2] = q1
    f = (q.astype(np.float32) - np.float32(2048.0)) \
        * np.float32(1.0 / np.float32(pack_scale))
    return q, f


class _Result:
    exec_time_ns = None


def kernel(**inputs):
    import jax

    x = np.asarray(inputs["node_feats"], np.float32)
    sc = np.asarray(inputs["sc"], np.float32)
    y = np.asarray(inputs["node_attrs"], np.float32)
    Wlin0 = np.asarray(inputs["Wlin0"], np.float32)
    Wlin1 = np.asarray(inputs["Wlin1"], np.float32)

    elem = np.argmax(y, axis=1)
    const_src = [np.asarray(inputs[k]) for k in (
        "U3_0", "U2_0", "U1_0", "W3_0", "W2_0", "W1_0",
        "U3_1", "U2_1", "U1_1", "W3_1", "W2_1", "W1_1")]

    # deal nodes: element e's nodes round-robin over cores; slots are
    # grouped per element but NOT block-aligned -- a block may span
    # element boundaries (handled by per-segment weighting ops)
    count = np.bincount(elem, minlength=E)
    spe = -(-count // NCORES)                    # slots used per core
    base_slot = np.zeros(E, np.int64)
    base_slot[1:] = np.cumsum(spe)[:-1]
    tot_slots = int(np.sum(spe))
    NBLK = -(-tot_slots // SLOTS_PER_BLK)
    NSLOT = NBLK * SLOTS_PER_BLK
    FT = NBLK * FB

    # per-block element segments (same on all cores); pad slots at the
    # very end are folded into the last element's segment (x there is 0)
    bounds = np.concatenate([base_slot, [NSLOT]])  # element e: [bounds[e], bounds[e+1])
    segs = []
    for b in range(NBLK):
        s0, s1 = b * SLOTS_PER_BLK, (b + 1) * SLOTS_PER_BLK
        bs = []
        for e in range(E):
            lo = max(s0, int(bounds[e]))
            hi = min(s1, int(bounds[e + 1] if e < E - 1 else NSLOT))
            if hi > lo:
                bs.append((e, lo - s0, hi - s0))
        segs.append(tuple(bs))
    segs = tuple(segs)

    order = np.argsort(elem, kind="stable")
    gstart = np.zeros(E, np.int64)
    gstart[1:] = np.cumsum(count)[:-1]
    j = np.arange(N) - gstart[elem[order]]
    core_of = np.empty(N, np.int64)
    slot_of = np.empty(N, np.int64)
    core_of[order] = j % NCORES
    slot_of[order] = base_slot[elem[order]] + j // NCORES

    key = segs
    if key not in _cache:
        nc = _build_nc(segs)
        ent = _make_dispatch(nc, FT)
        _cache[key] = ent
    ent = _cache[key]

    # xT in core-slot order: [core, i, slot, c] fp16
    x16 = x.astype(np.float16)
    XT9 = np.zeros((NCORES, I, NSLOT, C), np.float16)
    XT9[core_of, :, slot_of] = x16.transpose(0, 2, 1)
    xt_cat = XT9.reshape(NCORES * I, FT)

    # dispatch; the 12-bit pack scale self-settles: shrink on
    # saturation, then tighten once for precision (cached afterwards)
    for _ in range(12):
        dev_consts = _consts_device(ent, inputs, const_src)
        fbytes = _dispatch_once(ent, dev_consts, xt_cat)
        q, fdec = _decode_f(fbytes, ent["pack_scale"])
        qmax, qmin = int(q.max()), int(q.min())
        if qmax >= 4095 or qmin <= 0:
            ent["pack_scale"] = np.float16(float(ent["pack_scale"]) / 4.0)
            ent["settled"] = False
            continue
        if not ent["settled"]:
            fmax = max(float(np.abs(fdec).max()), 1e-6)
            s_opt = 2047.0 * 0.9 / fmax
            ent["settled"] = True
            if s_opt > 2.0 * float(ent["pack_scale"]):
                ent["pack_scale"] = np.float16(min(s_opt, 60000.0))
                continue
        break

    globals()["LAST_RESULT"] = _Result()
    nrep = int(os.environ.get("KERNEL_TIME_RUNS", "0"))
    if nrep:
        import time
        times = []
        for _ in range(nrep):
            t0 = time.perf_counter()
            dc = _consts_device(ent, inputs, const_src)
            _dispatch_once(ent, dc, xt_cat)
            times.append(time.perf_counter() - t0)
        globals()["LAST_TIMES"] = times

    f = fdec.reshape(NCORES, 4, NSLOT, C)
    f_ncd = f[core_of, :, slot_of]                      # [N, 4(dt), C] f32

    inv = np.float32(1.0 / np.sqrt(C))
    out = np.empty((N, C * 4), np.float32)
    out[:, :C] = (f_ncd[:, 0, :] @ Wlin0) * inv
    y1 = np.tensordot(f_ncd[:, 1:4, :], Wlin1, axes=([2], [0]))  # [N, d, w]
    out[:, C:] = (y1.transpose(0, 2, 1) * inv).reshape(N, 3 * C)
    out += sc
    return out
